# revision 1
# baseline (speedup 1.0000x reference)
"""Trainium2 Bass kernel for an FFM (field-aware factorization machine) forward pass.

Reference computation (all fp32):
    12 embedding matmuls over column slices of fv [32768, 2668], 15 pairwise
    dot-product cross terms, a linear layer and a sigmoid.

Restructuring: the 15 cross terms factor into 3 column-aligned block pairs
    cross = (mu+tu)·(ai+gi+oi+ui) + uu·(au+gu+ou) + mi·ti
            + au·(gu+ou) + gu·ou
so the whole model is 6 matmuls of fv @ W_block (W_block [2668, 128], built
host-side from the small tables), elementwise products of the 3 L/R pairs,
a partition-dim sum (ones-matmul), the linear term, bias and sigmoid.

Distribution: data-parallel over the batch dim — each of the 8 cores gets
4096 rows. The per-core feature matrix is transposed host-side so the device
streams [128-feature, batch] tiles straight into the PE array (contraction
dim on partitions) with no on-chip transposes. Matmuls run as float32r
(TF32-like, 1 PE cycle/row at N=512); inputs are pre-rounded to the fp32r
grid on the host, as the walrus verifier requires.
"""

import os
import numpy as np
from contextlib import ExitStack

B, F, D = 32768, 2668, 64
NCORES = 8
BL = B // NCORES          # batch rows per core
NKT = 21                  # feature K-tiles of 128
FP = NKT * 128            # padded feature dim (2688)
SUPER = 1024              # batch columns per DMA chunk
NSUB = 512                # matmul moving-dim (one fp32 PSUM bank)

BLOCK_NAMES = ("IL", "IR", "IIL", "IIR", "IIIL", "IIIR")
KTS = {
    "IL": tuple(range(7, 21)),
    "IR": tuple(range(0, 8)) + (20,),
    "IIL": tuple(range(0, 8)) + (20,),
    "IIR": (20,),
    "IIIL": (20,),
    "IIIR": (20,),
}
PAIRS = (("IL", "IR"), ("IIL", "IIR"), ("IIIL", "IIIR"))

# w_pack free-dim offsets: blocks (128 cols per K-tile), then lin (1 col per
# K-tile), then the ones column for the partition-sum reduce.
WOFF = {}
_off = 0
for _bn in BLOCK_NAMES:
    WOFF[_bn] = _off
    _off += 128 * len(KTS[_bn])
LIN_OFF = _off
_off += NKT
ONES_OFF = _off
WF = _off + 1
LIN_TILES = tuple(range(8, 20))  # t0..7 + t20 of lin ride the (IIL, IIR) pair

MM_DTYPE = os.environ.get("FFM_MM_DTYPE", "f32r")  # f32r | f32 | f16 | wf16


def _build_w_pack(inp):
    """Pack all block tables + lin_w + ones into one [128, WF] fp32 array laid
    out exactly as the SBUF weight tile wants it (partition k = row-in-K-tile)."""

    def z():
        return np.zeros((FP, D), np.float32)

    A_u, A_i = inp["age_user_w"], inp["age_item_w"]
    G_u, G_i = inp["gender_user_w"], inp["gender_item_w"]
    O_u, O_i = inp["occupation_user_w"], inp["occupation_item_w"]
    M_u, M_i = inp["movie_user_w"], inp["movie_item_w"]
    U_u, U_i = inp["userid_user_w"], inp["userid_item_w"]
    T_u, T_i = inp["itemid_user_w"], inp["itemid_item_w"]

    MT = z(); MT[943:2625] = T_u; MT[2649:2668] = M_u              # mu + tu
    TI = z(); TI[943:2625] = T_i                                    # ti
    S = z(); S[0:943] = U_i; S[2626:2627] += A_i
    S[2626:2628] += G_i; S[2628:2649] += O_i                        # ai+gi+oi+ui
    MI = z(); MI[2649:2668] = M_i                                   # mi
    UU = z(); UU[0:943] = U_u                                       # uu
    AU = z(); AU[2626:2627] = A_u                                   # au
    R = z(); R[2626:2627] += A_u; R[2626:2628] += G_u
    R[2628:2649] += O_u                                             # au+gu+ou
    GUOU = z(); GUOU[2626:2628] += G_u; GUOU[2628:2649] += O_u      # gu+ou
    GU = z(); GU[2626:2628] = G_u
    OU = z(); OU[2628:2649] = O_u
    Z = np.zeros((FP, D), np.float32)

    lw = np.zeros(FP, np.float32)
    lw[:F] = np.asarray(inp["lin_w"], np.float32)[0]
    # lin_B: the part of lin_w living in block II's K-tiles (t0..7, t20) rides
    # as column 64 of the (IIL, IIR) pair — the partner column in IIR selects
    # the host-injected ones-row (fv row 2668 == 1.0), making that product
    # column exactly lin_B. Only t8..19 keep the dedicated M=1 lin chain.
    LWB = np.zeros((FP, 1), np.float32)
    for _t in KTS["IIL"]:
        LWB[_t * 128:(_t + 1) * 128, 0] = lw[_t * 128:(_t + 1) * 128]
    E1 = np.zeros((FP, 1), np.float32)
    E1[F, 0] = 1.0  # selects the ones-feature row
    Z63 = np.zeros((FP, 63), np.float32)

    blk = {
        "IL": np.hstack([MT, TI]),
        "IR": np.hstack([S, MI]),
        "IIL": np.hstack([UU, LWB, Z63]),
        "IIR": np.hstack([R, E1, Z63]),
        "IIIL": np.hstack([GU, AU]),
        "IIIR": np.hstack([OU, GUOU]),
    }

    w_pack = np.zeros((128, WF), np.float32)
    for bn in BLOCK_NAMES:
        W = blk[bn]
        for j, t in enumerate(KTS[bn]):
            w_pack[:, WOFF[bn] + j * 128:WOFF[bn] + (j + 1) * 128] = \
                W[t * 128:(t + 1) * 128]
    for t in range(NKT):
        w_pack[:, LIN_OFF + t] = lw[t * 128:(t + 1) * 128]
    w_pack[:, ONES_OFF] = 1.0
    return w_pack


def _trace_kernel(ctx: ExitStack, tc, out_d, fvt_d, w_d, lb_d, mm_dt, w_dt,
                  onesr_d=None, repeat=1, loop=False, skip_lin=False,
                  lin_dve=False):
    import concourse.mybir as mybir

    nc = tc.nc
    f32 = mybir.dt.float32

    blocks_at_kt = [[bn for bn in BLOCK_NAMES if t in KTS[bn]]
                    for t in range(NKT)]

    wpool = ctx.enter_context(tc.tile_pool(name="wpool", bufs=1))
    w_sb = wpool.tile([128, WF], w_dt, name="w_sb")
    # Load weights hottest-first so the first matmuls aren't gated on the
    # whole 2.3 MB: the two 64 KB K-tile-0 slices of IR/IIL, then the rest of
    # the t0-needed region, then IL (first needed at K-tile 7).
    il_end = WOFF["IR"]
    for lo, hi in ((WOFF["IR"], WOFF["IR"] + 128),
                   (WOFF["IIL"], WOFF["IIL"] + 128),
                   (WOFF["IR"] + 128, WOFF["IIL"]),
                   (WOFF["IIL"] + 128, WF),
                   (0, il_end)):
        nc.sync.dma_start(w_sb[:, lo:hi], w_d[:, lo:hi])
    lb_sb = wpool.tile([1, 1], f32, name="lb_sb")
    nc.sync.dma_start(lb_sb[:], lb_d[:])

    fpool = ctx.enter_context(tc.tile_pool(name="fpool", bufs=38))
    pspool = ctx.enter_context(tc.tile_pool(name="pspool", bufs=1, space="PSUM"))
    prodpool = ctx.enter_context(tc.tile_pool(name="prodpool", bufs=3))
    opool = ctx.enter_context(tc.tile_pool(name="opool", bufs=2))

    if w_dt == mybir.dt.float16:
        # fp16 weights can't feed the f32r ones-reduce; DMA a separate f32r
        # ones vector (memset can't write f32r)
        r_dt = mybir.dt.float32r
        ones_sb = wpool.tile([128, 1], r_dt, name="ones_sb")
        nc.sync.dma_start(ones_sb[:], onesr_d[:])
        ones_ap = ones_sb[:]
    else:
        r_dt = mm_dt
        ones_ap = w_sb[:, ONES_OFF:ONES_OFF + 1]

    def _body(rep):
        for s in range(BL // SUPER):
            fts = []
            for t in range(NKT):
                ft = fpool.tile([128, SUPER], mm_dt, tag="fvt",
                                name=f"fvt_{rep}_{s}_{t}")
                # alternate the two HWDGE rings (SP / ACT) so descriptor
                # generation for the streaming loads isn't single-ring bound
                if os.environ.get("FFM_SWDGE") == "1":
                    eng = nc.gpsimd
                else:
                    eng = nc.sync if t % 2 == 0 else nc.scalar
                eng.dma_start(
                    ft[:],
                    fvt_d[t * 128:(t + 1) * 128,
                          s * SUPER:(s + 1) * SUPER])
                fts.append(ft)
            for sub in range(SUPER // NSUB):
                ps = {}
                for bn in BLOCK_NAMES:
                    ps[bn] = pspool.tile([128, NSUB], f32, tag=f"ps_{bn}",
                                         name=f"ps_{bn}_{rep}_{s}_{sub}")
                logit = pspool.tile([1, NSUB], f32, tag="logit", bufs=2,
                                    name=f"logit_{rep}_{s}_{sub}")
                accs = [None, None]  # two parities to halve the dep chain
                for t in range(NKT):
                    rhs = fts[t][:, sub * NSUB:(sub + 1) * NSUB]
                    for bn in blocks_at_kt[t]:
                        kts = KTS[bn]
                        off = WOFF[bn] + kts.index(t) * 128
                        nc.tensor.matmul(
                            ps[bn][:], w_sb[:, off:off + 128], rhs,
                            start=(t == kts[0]), stop=(t == kts[-1]))
                    if lin_dve:
                        # linear term on DVE: per-partition-scalar mult of the
                        # resident fv tile, chained accumulate in fp32
                        rhs32 = rhs.bitcast(f32)
                        w32 = w_sb[:, LIN_OFF + t:LIN_OFF + t + 1].bitcast(f32)
                        par = t % 2
                        if accs[par] is None:
                            at = prodpool.tile([128, NSUB], f32, tag=f"acc{par}",
                                               bufs=2,
                                               name=f"acc{par}_{rep}_{s}_{sub}")
                            nc.vector.tensor_single_scalar(
                                at[:], rhs32, w32, mybir.AluOpType.mult)
                            accs[par] = at
                        else:
                            nc.vector.scalar_tensor_tensor(
                                accs[par][:], rhs32, w32, accs[par][:],
                                mybir.AluOpType.mult, mybir.AluOpType.add)
                    elif not skip_lin and t in LIN_TILES:
                        nc.tensor.matmul(
                            logit[:],
                            w_sb[:, LIN_OFF + t:LIN_OFF + t + 1], rhs,
                            start=(t == LIN_TILES[0]), stop=False)
                prods = []
                for pl, pr in PAIRS:
                    # the ISA allows at most one PSUM source per
                    # tensor_tensor, so drain the L operand to SBUF first
                    lt = prodpool.tile([128, NSUB], f32, tag="ldrain",
                                       name=f"ldrain_{pl}_{rep}_{s}_{sub}")
                    if lin_dve:
                        nc.scalar.copy(lt[:], ps[pl][:])
                    else:
                        nc.vector.tensor_copy(lt[:], ps[pl][:])
                    pt = prodpool.tile([128, NSUB], r_dt, tag="prod", bufs=4,
                                       name=f"prod_{pl}_{rep}_{s}_{sub}")
                    nc.vector.tensor_mul(pt[:], lt[:], ps[pr][:])
                    prods.append(pt)
                if lin_dve:
                    p4 = prodpool.tile([128, NSUB], r_dt, tag="prod", bufs=4,
                                       name=f"prod_lin_{rep}_{s}_{sub}")
                    nc.vector.tensor_add(p4[:], accs[0][:], accs[1][:])
                    prods.append(p4)
                first_start = skip_lin or lin_dve
                for j, pt in enumerate(prods):
                    nc.tensor.matmul(logit[:], ones_ap, pt[:],
                                     start=(first_start and j == 0),
                                     stop=(j == len(prods) - 1))
                out_sb = opool.tile([1, NSUB], f32, tag="out",
                                    name=f"out_{rep}_{s}_{sub}")
                nc.scalar.activation(out_sb[:], logit[:],
                                     mybir.ActivationFunctionType.Sigmoid,
                                     bias=lb_sb[0:1, 0:1], scale=1.0)
                col = s * SUPER + sub * NSUB
                nc.scalar.dma_start(out_d[0:1, col:col + NSUB], out_sb[:])

    if loop and repeat > 1:
        # benchmarking mode: run the identical body `repeat` times inside one
        # NEFF via a hardware loop (one dispatch, `repeat` full passes)
        with tc.For_i(0, repeat, 1):
            _body(0)
    else:
        for rep in range(repeat):
            _body(rep)


_MODULES = {}


def get_module(repeat=1, loop=False, skip_lin=False, lin_dve=False):
    """Build (once per config) and return the compiled Bass module."""
    key = (repeat, loop, skip_lin, lin_dve)
    if key in _MODULES:
        return _MODULES[key]

    import concourse.bacc as bacc
    import concourse.tile as tile
    import concourse.mybir as mybir

    mm_dt = {"f32r": mybir.dt.float32r, "f32": mybir.dt.float32,
             "f16": mybir.dt.float16, "wf16": mybir.dt.float32r}[MM_DTYPE]
    w_dt = mybir.dt.float16 if MM_DTYPE in ("f16", "wf16") else mm_dt

    nc = bacc.Bacc("TRN2", debug=False, enable_asserts=False,
                   num_devices=NCORES)
    fvt_d = nc.dram_tensor("fvt", (FP, BL), mm_dt,
                           kind="ExternalInput").ap()
    w_d = nc.dram_tensor("wpack", (128, WF), w_dt,
                         kind="ExternalInput").ap()
    lb_d = nc.dram_tensor("linb", (1, 1), mybir.dt.float32,
                          kind="ExternalInput").ap()
    onesr_d = None
    if MM_DTYPE in ("f16", "wf16"):
        onesr_d = nc.dram_tensor("onesr", (128, 1), mybir.dt.float32r,
                                 kind="ExternalInput").ap()
    out_d = nc.dram_tensor("out", (1, BL), mybir.dt.float32,
                           kind="ExternalOutput").ap()

    with tile.TileContext(nc) as tc, ExitStack() as ctx:
        _trace_kernel(ctx, tc, out_d, fvt_d, w_d, lb_d, mm_dt, w_dt,
                      onesr_d=onesr_d, repeat=repeat, loop=loop,
                      skip_lin=skip_lin, lin_dve=lin_dve)
    nc.compile()
    _MODULES[key] = nc
    return nc


def _to_f32r(x):
    from neuron_dtypes import static_cast_fp32_to_fp32r
    return np.ascontiguousarray(
        static_cast_fp32_to_fp32r(np.ascontiguousarray(x))
    ).view(np.float32).reshape(x.shape)


def _round_fv(x):
    if MM_DTYPE == "f16":
        return np.ascontiguousarray(x, np.float16)
    if MM_DTYPE in ("f32r", "wf16"):
        return _to_f32r(x)
    return x


def _round_w(x):
    if MM_DTYPE in ("f16", "wf16"):
        return np.ascontiguousarray(x, np.float16)
    if MM_DTYPE == "f32r":
        return _to_f32r(x)
    return x


def prepare_in_maps(inputs):
    """Host-side sharding: batch-split fv, transpose each shard to
    feature-major (padded to 2688 rows), replicate the packed weights."""
    fv = np.ascontiguousarray(np.asarray(inputs["feature_vector"], np.float32))
    assert fv.shape == (B, F)
    w_pack = _round_w(_build_w_pack({k: np.asarray(v, np.float32)
                                     for k, v in inputs.items()
                                     if k != "feature_vector"}))
    lb = np.asarray(inputs["lin_b"], np.float32).reshape(1, 1)

    in_maps = []
    for c in range(NCORES):
        fvt = np.zeros((FP, BL), np.float32)
        fvt[:F] = fv[c * BL:(c + 1) * BL].T
        fvt[F] = 1.0  # ones-feature row pairing with lin_B in block II
        m = {"fvt": _round_fv(fvt), "wpack": w_pack, "linb": lb}
        if MM_DTYPE in ("f16", "wf16"):
            m["onesr"] = np.ones((128, 1), np.float32)
        in_maps.append(m)
    return in_maps


def kernel(**inputs) -> np.ndarray:
    # Tracing needs the axon NTFF hook, which this environment lacks; make
    # sure a stray BASS_TRACE=1 can't crash the run.
    os.environ["BASS_NEVER_TRACE"] = "1"
    from concourse import bass_utils

    in_maps = prepare_in_maps(inputs)
    nc = get_module()
    try:
        res = bass_utils.run_bass_kernel_spmd(nc, in_maps,
                                              core_ids=list(range(NCORES)))
    except Exception:
        # transient NRT device errors have been observed on this fabric;
        # one retry after a short pause usually succeeds
        import time
        time.sleep(15)
        res = bass_utils.run_bass_kernel_spmd(nc, in_maps,
                                              core_ids=list(range(NCORES)))
    out = np.concatenate([r["out"].reshape(BL) for r in res.results])
    return out.reshape(B, 1).astype(np.float32)



# revision 10
# speedup vs baseline: 1.7350x; 1.7350x over previous
"""Trainium2 Bass kernel for an FFM (field-aware factorization machine) forward pass.

Reference computation (all fp32):
    12 embedding matmuls over column slices of fv [32768, 2668], 15 pairwise
    dot-product cross terms, a linear layer and a sigmoid.

Restructuring: the 15 cross terms factor into 5 column-aligned products
    cross = (mu+tu)·(ai+gi+oi+ui) + mi·ti + uu·(au+gu+ou)
            + au·(gu+ou) + gu·ou
packed into 3 PSUM pairs (64-col halves):
    P1 = [MT|TI] x [S |MI]   L: t7..20 (14 mm)   R: t0..7,t20 (9 mm)
    P2 = [UU|lw] x [R4|e1]   L: t0..7   (8 mm)   R: t20       (1 mm)
    P3 = [GU|AU] x [OU|GO]   L: t20     (1 mm)   R: t20       (1 mm)
P2's column 64 carries the linear term for K-tiles 0..7 (lw column paired
with a ones-row selector); the remaining linear tiles t8..20 accumulate on
the Vector engine as per-partition-scalar multiply-adds in fp16 (2x mode).
Pair products and their sum run on DVE; PSUM->SBUF drains and the final
sigmoid on the Act engine; a single ones-matmul does the partition-sum.

Distribution: data-parallel over the batch dim - each of the 8 cores gets
4096 rows. The per-core feature matrix is transposed host-side to fp16
[feature, batch] so the device streams [128, batch] tiles straight into
the PE array (contraction dim on partitions) with no on-chip transposes.
fp16 halves the HBM traffic vs fp32 (the kernel is memory-bound) and its
quantization error (~2e-3 rel on the sigmoid output) sits far below the
2e-2 gate.
"""

import os
import numpy as np
from contextlib import ExitStack

B, F, D = 32768, 2668, 64
NCORES = 8
BL = B // NCORES          # batch rows per core
NKT = 21                  # feature K-tiles of 128
FP = NKT * 128            # padded feature dim (2688)
SUPER = 1024              # batch columns per DMA chunk
NSUB = 512                # matmul moving-dim (one fp32 PSUM bank)

# pair sides and the K-tiles each accumulates over
SIDES = ("P1L", "P1R", "P2L", "P2R", "P3L", "P3R")
KTS = {
    "P1L": tuple(range(7, 21)),
    "P1R": tuple(range(0, 8)) + (20,),
    "P2L": tuple(range(0, 8)),
    "P2R": (20,),
    "P3L": (20,),
    "P3R": (20,),
}
PAIRS = (("P1L", "P1R"), ("P2L", "P2R"), ("P3L", "P3R"))
LIN_TILES = tuple(range(8, 21))   # linear-term K-tiles handled on DVE

# w_pack free-dim offsets: one 128-col slice per (side, tile), then the DVE
# linear scalars (1 col per lin tile), then an fp16 ones column.
WOFF = {}
_off = 0
for _sn in SIDES:
    WOFF[_sn] = _off
    _off += 128 * len(KTS[_sn])
ONES_OFF = _off
WF = _off + 1
NLIN = len(LIN_TILES)


def _build_w_pack(inp):
    """Pack all pair-side tables + DVE lin scalars + ones into one [128, WF]
    fp16 array laid out exactly as the SBUF weight tile wants it."""

    def z():
        return np.zeros((FP, D), np.float32)

    A_u, A_i = inp["age_user_w"], inp["age_item_w"]
    G_u, G_i = inp["gender_user_w"], inp["gender_item_w"]
    O_u, O_i = inp["occupation_user_w"], inp["occupation_item_w"]
    M_u, M_i = inp["movie_user_w"], inp["movie_item_w"]
    U_u, U_i = inp["userid_user_w"], inp["userid_item_w"]
    T_u, T_i = inp["itemid_user_w"], inp["itemid_item_w"]

    MT = z(); MT[943:2625] = T_u; MT[2649:2668] = M_u              # mu + tu
    TI = z(); TI[943:2625] = T_i                                    # ti
    S = z(); S[0:943] = U_i; S[2626:2627] += A_i
    S[2626:2628] += G_i; S[2628:2649] += O_i                        # ai+gi+oi+ui
    MI = z(); MI[2649:2668] = M_i                                   # mi
    UU = z(); UU[0:943] = U_u                                       # uu
    R4 = z(); R4[2626:2627] += A_u; R4[2626:2628] += G_u
    R4[2628:2649] += O_u                                            # au+gu+ou
    AU = z(); AU[2626:2627] = A_u                                   # au
    GU = z(); GU[2626:2628] = G_u                                   # gu
    OU = z(); OU[2628:2649] = O_u                                   # ou
    GO = z(); GO[2626:2628] += G_u; GO[2628:2649] += O_u            # gu+ou

    lw = np.zeros(FP, np.float32)
    lw[:F] = np.asarray(inp["lin_w"], np.float32)[0]
    # linear term for t0..7 rides P2 column 64: (fv @ LWB) * (fv @ E1) where
    # E1 selects the host-injected ones-row (fvt row 2668 == 1.0)
    LWB = np.zeros((FP, 1), np.float32)
    LWB[:1024, 0] = lw[:1024]
    E1 = np.zeros((FP, 1), np.float32)
    E1[F, 0] = 1.0
    Z63 = np.zeros((FP, 63), np.float32)

    mats = {
        "P1L": np.hstack([MT, TI]),
        "P1R": np.hstack([S, MI]),
        "P2L": np.hstack([UU, LWB, Z63]),
        "P2R": np.hstack([R4, E1, Z63]),
        "P3L": np.hstack([GU, AU]),
        "P3R": np.hstack([OU, GO]),
    }

    w_pack = np.zeros((128, WF), np.float32)
    for sn in SIDES:
        W = mats[sn]
        for j, t in enumerate(KTS[sn]):
            w_pack[:, WOFF[sn] + j * 128:WOFF[sn] + (j + 1) * 128] = \
                W[t * 128:(t + 1) * 128]
    w_pack[:, ONES_OFF] = 1.0
    # DVE lin scalars stay fp32 (the ISA requires a float32 scalar operand)
    lin_w = np.zeros((128, NLIN), np.float32)
    for j, t in enumerate(LIN_TILES):
        lin_w[:, j] = lw[t * 128:(t + 1) * 128]
    return w_pack.astype(np.float16), lin_w


def _trace_kernel(ctx: ExitStack, tc, out_d, fvt_d, w_d, lb_d, onesr_d,
                  linw_d, repeat=1, loop=False):
    import concourse.mybir as mybir

    nc = tc.nc
    f32 = mybir.dt.float32
    f32r = mybir.dt.float32r
    f16 = mybir.dt.float16

    sides_at_kt = [[sn for sn in SIDES if t in KTS[sn]] for t in range(NKT)]

    wpool = ctx.enter_context(tc.tile_pool(name="wpool", bufs=1))
    w_sb = wpool.tile([128, WF], f16, name="w_sb")
    # Load weights hottest-first so the first matmuls aren't gated on the
    # whole pack: the K-tile-0 slices of P1R/P2L first, then the rest.
    for lo, hi in ((WOFF["P1R"], WOFF["P1R"] + 128),
                   (WOFF["P2L"], WOFF["P2L"] + 128),
                   (WOFF["P1R"] + 128, WF),
                   (0, WOFF["P1R"])):
        nc.sync.dma_start(w_sb[:, lo:hi], w_d[:, lo:hi])
    lb_sb = wpool.tile([1, 1], f32, name="lb_sb")
    nc.sync.dma_start(lb_sb[:], lb_d[:])
    # f32r ones vector for the partition-sum reduce (memset can't write f32r)
    ones_sb = wpool.tile([128, 1], f32r, name="ones_sb")
    nc.sync.dma_start(ones_sb[:], onesr_d[:])
    linw_sb = wpool.tile([128, NLIN], f32, name="linw_sb")
    nc.sync.dma_start(linw_sb[:], linw_d[:])

    fpool = ctx.enter_context(tc.tile_pool(name="fpool", bufs=38))
    pspool = ctx.enter_context(tc.tile_pool(name="pspool", bufs=1, space="PSUM"))
    dpool = ctx.enter_context(tc.tile_pool(name="dpool", bufs=6))
    mpool = ctx.enter_context(tc.tile_pool(name="mpool", bufs=2))
    opool = ctx.enter_context(tc.tile_pool(name="opool", bufs=2))

    def _body(rep):
        for s in range(BL // SUPER):
            fts = []
            for t in range(NKT):
                ft = fpool.tile([128, SUPER], f16, tag="fvt",
                                name=f"fvt_{rep}_{s}_{t}")
                # alternate the two HWDGE rings (SP / ACT) so descriptor
                # generation for the streaming loads isn't single-ring bound
                eng = nc.sync if t % 2 == 0 else nc.scalar
                eng.dma_start(
                    ft[:],
                    fvt_d[t * 128:(t + 1) * 128,
                          s * SUPER:(s + 1) * SUPER])
                fts.append(ft)
            for sub in range(SUPER // NSUB):
                ps = {}
                for sn in SIDES:
                    ps[sn] = pspool.tile([128, NSUB], f32, tag=f"ps_{sn}",
                                         name=f"ps_{sn}_{rep}_{s}_{sub}")
                logit = pspool.tile([1, NSUB], f32, tag="logit", bufs=2,
                                    name=f"logit_{rep}_{s}_{sub}")
                acc = None
                for t in range(NKT):
                    rhs = fts[t][:, sub * NSUB:(sub + 1) * NSUB]
                    for sn in sides_at_kt[t]:
                        kts = KTS[sn]
                        off = WOFF[sn] + kts.index(t) * 128
                        nc.tensor.matmul(
                            ps[sn][:], w_sb[:, off:off + 128], rhs,
                            start=(t == kts[0]), stop=(t == kts[-1]))
                    if t in LIN_TILES:
                        # linear term on DVE: per-partition-scalar mult of the
                        # resident fv tile, chained accumulate in fp16 (2x mode)
                        j = LIN_TILES.index(t)
                        wcol = linw_sb[:, j:j + 1]
                        if acc is None:
                            acc = mpool.tile([128, NSUB], f16, tag="acc",
                                             name=f"acc_{rep}_{s}_{sub}")
                            nc.vector.tensor_single_scalar(
                                acc[:], rhs, wcol, mybir.AluOpType.mult)
                        else:
                            nc.vector.scalar_tensor_tensor(
                                acc[:], rhs, wcol, acc[:],
                                mybir.AluOpType.mult, mybir.AluOpType.add)
                prods = []
                for pl, pr in PAIRS:
                    # the ISA allows at most one PSUM source per tensor_tensor,
                    # so drain the L operand to SBUF on the Act engine first
                    lt = dpool.tile([128, NSUB], f32, tag=f"ld_{pl}",
                                    name=f"ld_{pl}_{rep}_{s}_{sub}")
                    nc.scalar.copy(lt[:], ps[pl][:])
                    pt = mpool.tile([128, NSUB], f32r, tag=f"pm_{pl}",
                                    name=f"pm_{pl}_{rep}_{s}_{sub}")
                    nc.vector.tensor_mul(pt[:], lt[:], ps[pr][:])
                    prods.append(pt)
                s1 = mpool.tile([128, NSUB], f32r, tag="s1",
                                name=f"s1_{rep}_{s}_{sub}")
                nc.vector.tensor_add(s1[:], prods[0][:], prods[1][:])
                s2 = mpool.tile([128, NSUB], f32r, tag="s2",
                                name=f"s2_{rep}_{s}_{sub}")
                nc.vector.tensor_add(s2[:], s1[:], prods[2][:])
                s3 = mpool.tile([128, NSUB], f32r, tag="s3",
                                name=f"s3_{rep}_{s}_{sub}")
                nc.vector.tensor_add(s3[:], s2[:], acc[:])
                nc.tensor.matmul(logit[:], ones_sb[:], s3[:],
                                 start=True, stop=True)
                out_sb = opool.tile([1, NSUB], f32, tag="out",
                                    name=f"out_{rep}_{s}_{sub}")
                nc.scalar.activation(out_sb[:], logit[:],
                                     mybir.ActivationFunctionType.Sigmoid,
                                     bias=lb_sb[0:1, 0:1], scale=1.0)
                col = s * SUPER + sub * NSUB
                nc.scalar.dma_start(out_d[0:1, col:col + NSUB], out_sb[:])

    if loop and repeat > 1:
        # benchmarking mode: run the identical body `repeat` times inside one
        # NEFF via a hardware loop (one dispatch, `repeat` full passes)
        with tc.For_i(0, repeat, 1):
            _body(0)
    else:
        for rep in range(repeat):
            _body(rep)


_MODULES = {}


def get_module(repeat=1, loop=False, **_unused):
    """Build (once per config) and return the compiled Bass module."""
    key = (repeat, loop)
    if key in _MODULES:
        return _MODULES[key]

    import concourse.bacc as bacc
    import concourse.tile as tile
    import concourse.mybir as mybir

    nc = bacc.Bacc("TRN2", debug=False, enable_asserts=False,
                   num_devices=NCORES)
    fvt_d = nc.dram_tensor("fvt", (FP, BL), mybir.dt.float16,
                           kind="ExternalInput").ap()
    w_d = nc.dram_tensor("wpack", (128, WF), mybir.dt.float16,
                         kind="ExternalInput").ap()
    lb_d = nc.dram_tensor("linb", (1, 1), mybir.dt.float32,
                          kind="ExternalInput").ap()
    onesr_d = nc.dram_tensor("onesr", (128, 1), mybir.dt.float32r,
                             kind="ExternalInput").ap()
    linw_d = nc.dram_tensor("linw", (128, NLIN), mybir.dt.float32,
                            kind="ExternalInput").ap()
    out_d = nc.dram_tensor("out", (1, BL), mybir.dt.float32,
                           kind="ExternalOutput").ap()

    with tile.TileContext(nc) as tc, ExitStack() as ctx:
        _trace_kernel(ctx, tc, out_d, fvt_d, w_d, lb_d, onesr_d, linw_d,
                      repeat=repeat, loop=loop)
    nc.compile()
    _MODULES[key] = nc
    return nc


def prepare_in_maps(inputs):
    """Host-side sharding: batch-split fv, transpose each shard to fp16
    feature-major (padded to 2688 rows), replicate the packed weights."""
    fv = np.asarray(inputs["feature_vector"], np.float32)
    assert fv.shape == (B, F)
    w_pack, lin_w = _build_w_pack({k: np.asarray(v, np.float32)
                                   for k, v in inputs.items()
                                   if k != "feature_vector"})
    lb = np.asarray(inputs["lin_b"], np.float32).reshape(1, 1)
    onesr = np.ones((128, 1), np.float32)

    fv16 = fv.astype(np.float16)
    in_maps = []
    for c in range(NCORES):
        fvt = np.zeros((FP, BL), np.float16)
        fvt[:F] = fv16[c * BL:(c + 1) * BL].T
        fvt[F] = 1.0  # ones-feature row pairing with the lin column in P2
        in_maps.append({"fvt": fvt, "wpack": w_pack, "linb": lb,
                        "onesr": onesr, "linw": lin_w})
    return in_maps


def kernel(**inputs) -> np.ndarray:
    # Tracing needs the axon NTFF hook, which this environment lacks; make
    # sure a stray BASS_TRACE=1 can't crash the run.
    os.environ["BASS_NEVER_TRACE"] = "1"
    from concourse import bass_utils

    in_maps = prepare_in_maps(inputs)
    nc = get_module()
    try:
        res = bass_utils.run_bass_kernel_spmd(nc, in_maps,
                                              core_ids=list(range(NCORES)))
    except Exception:
        # transient NRT device errors have been observed on this fabric;
        # one retry after a short pause usually succeeds
        import time
        time.sleep(15)
        res = bass_utils.run_bass_kernel_spmd(nc, in_maps,
                                              core_ids=list(range(NCORES)))
    out = np.concatenate([r["out"].reshape(BL) for r in res.results])
    return out.reshape(B, 1).astype(np.float32)
